# revision 1
# baseline (speedup 1.0000x reference)
"""Trainium2 Bass kernel for the DeformableDetr sparse-attention module.

Reference semantics (single device):
    q   = query.transpose(1,0,2)              # [bs, nq, c]
    attn = softmax((q @ W_attn + b_attn).reshape(bs,nq,H,P), -1)
    v    = memory[0] @ W_val + b_val          # only memory token 0 is live
    out  = (attn.sum(-1)[...,None] * v.reshape(bs,1,H,dh)).reshape(bs,nq,c)
    out  = out @ W_out + b_out
    return out.transpose(1,0,2)               # [nq, bs, c]

Only memory[0] feeds the output (the reference computes the full value
projection but keeps values[:, 0]), and attn.sum(-1) is a softmax summed
over its own axis.  The kernel computes the live math faithfully on
device:  s = S * (1/S) with S the per-head exp-sum (exact for the given
zero attention weights; the logits branch runs in bf16, which cannot
perturb s because S and 1/S come from the same f32 PSUM value), then
out_b^T[c',n] = sum_h U_b[h,c'] s_b[h,n] + b_out[c'] where
U_b[h,:] = v_b[h*32:(h+1)*32] @ W_out[h*32:(h+1)*32, :] and the b_out
term rides the PSUM->SBUF output copies as a per-partition bias.

This walrus build rejects instructions carrying more than one sync wait;
_split_multiwaits() legalizes the module by moving excess waits onto
same-engine InstNoOps placed directly before the instruction (the
in-order sequencer stalls on each semaphore in turn -- semantically
identical).

Sharding: data-parallel over batch, 2 batch elements per core x 8 cores.
"""

import sys

import numpy as np

sys.path.insert(0, "/opt/trn_rl_repo")

import ml_dtypes

import concourse.bass as bass
import concourse.tile as tile
from concourse import mybir
from concourse.bass_utils import run_bass_kernel_spmd

NQ, BS, NS, D = 300, 16, 13294, 256
N_HEADS, N_POINTS = 8, 4
N_CORES = 8
BPC = BS // N_CORES          # batch elements per core
J = NQ * BPC                 # query columns per core
F32 = mybir.dt.float32
F32R = mybir.dt.float32r
BF16 = mybir.dt.bfloat16
BF = ml_dtypes.bfloat16

# wpA: f32 value-projection panel [128, 518]
A_WVAL = 0                   # [128, 512], col 256*k + c'
A_M0T = A_WVAL + 512         # [128, 4], col 2*k + b
A_BVAL = A_M0T + 2 * BPC     # [128, 2], col m = b_val[128m:128(m+1)]
A_COLS = A_BVAL + 2          # = 518

# wpB: f32 output-projection panel [128, 514]
B_WOUT = 0                   # [128, 512], col 256*k + c2
B_BOUT = B_WOUT + 512        # [128, 2], col m = b_out[128m:128(m+1)]
B_COLS = B_BOUT + 2          # = 514

# bf16 attention-constant panel (tiny, lands first) and query panels;
# qd<b> holds cols 300k+n = qT[128k+p, b, n].
QA_WATTN = 0                 # [128, 64] W_attn k-major
QA_G0 = QA_WATTN + 64        # [32, 40] G at cols 0..31 (4 copies), 0 at 32..39
QA_G1 = QA_G0 + 40           # [32, 40] 0 at cols 0..31, G at 32..39
QA_BATTN = QA_G1 + 40        # [32, 1] b_attn (bf16; only perturbs the exp
                             # whose sum is self-normalized, s = S*(1/S))
QA_COLS = QA_BATTN + 1       # = 145
QD_COLS = J                  # = 600

_BASS_CACHE: dict = {}


def _split_multiwaits(nc: bass.Bass) -> None:
    for fn in nc.m.functions:
        for blk in fn.blocks:
            out, changed = [], False
            for inst in blk.instructions:
                si = inst.sync_info
                if si is not None and len(si.on_wait) > 1:
                    waits = list(si.on_wait)
                    for i, w in enumerate(waits[:-1]):
                        out.append(
                            mybir.InstNoOp(
                                name=f"{inst.name}_prewait{i}",
                                engine=inst.engine,
                                bass_nofuse=True,
                                sync_info=mybir.SyncInfo(on_wait=[w], on_update=[]),
                            )
                        )
                    inst.sync_info = mybir.SyncInfo(
                        on_wait=[waits[-1]], on_update=list(si.on_update)
                    )
                    changed = True
                out.append(inst)
            if changed:
                blk.instructions = out


def _build_bass(split: bool = True) -> bass.Bass:
    nc = bass.Bass()
    wpA = nc.declare_dram_parameter("wpA", [128, A_COLS], F32, isOutput=False)
    wpB = nc.declare_dram_parameter("wpB", [128, B_COLS], F32, isOutput=False)
    qattn = nc.declare_dram_parameter("qattn", [128, QA_COLS], BF16, isOutput=False)
    qd0 = nc.declare_dram_parameter("qd0", [128, QD_COLS], BF16, isOutput=False)
    qd1 = nc.declare_dram_parameter("qd1", [128, QD_COLS], BF16, isOutput=False)
    out2 = nc.declare_dram_parameter("out2", [128, J * 2], F32, isOutput=True)

    ACT = mybir.ActivationFunctionType

    with tile.TileContext(nc) as tc:
        with (
            tc.tile_pool(name="consts", bufs=1) as cp,
            tc.tile_pool(name="work", bufs=2) as wp,
            tc.tile_pool(name="ps", bufs=2, space="PSUM") as ps,
        ):
            # ---- loads. SP/HWDGE (lower first-byte latency) carries the
            # critical-path attention inputs and wpB; Pool/SWDGE carries the
            # rest in parallel.
            qd_sb = [cp.tile([128, QD_COLS], BF16, name="qd0_sb"),
                     cp.tile([128, QD_COLS], BF16, name="qd1_sb")]
            nc.sync.dma_start(out=qd_sb[0], in_=qd0[:, :])
            nc.sync.dma_start(out=qd_sb[1], in_=qd1[:, :])
            wpB_sb = cp.tile([128, B_COLS], F32)
            nc.sync.dma_start(out=wpB_sb, in_=wpB[:, :])
            qattn_sb = cp.tile([128, QA_COLS], BF16)
            nc.gpsimd.dma_start(out=qattn_sb, in_=qattn[:, :])
            wpA_sb = cp.tile([128, A_COLS], F32)
            nc.gpsimd.dma_start(out=wpA_sb, in_=wpA[:, :])

            # warm the ACT exp table while DMAs stream
            warm_sb = cp.tile([1, 1], F32)
            nc.scalar.activation(out=warm_sb, in_=nc.const_aps.tensor(0.0, (1, 1)),
                                 func=ACT.Exp)

            # ---- attention branch: logits^T -> exp -> fused head-group sums
            # s40 rows 0-7: s for b=0; rows 32-39: s for b=1 (cols = query n)
            E_act = wp.tile([32, J], BF16, bufs=1)
            ps_lgs = []
            for half in range(2):
                ps_lg = ps.tile([32, NQ], F32, tag="lg")
                for k in range(2):
                    nc.tensor.matmul(
                        ps_lg,
                        qattn_sb[:, QA_WATTN + 32 * k:QA_WATTN + 32 * (k + 1)],
                        qd_sb[half][:, NQ * k:NQ * (k + 1)],
                        start=(k == 0),
                        stop=(k == 1),
                    )
                ps_lgs.append(ps_lg)

            # ---- value branch: v^T = W_val^T @ m0^T + b_val (copies on DVE)
            v_sb = []
            for m in range(2):
                ps_v = ps.tile([128, BPC], F32, tag="v", bufs=1)
                for k in range(2):
                    base = A_WVAL + 256 * k + 128 * m
                    nc.tensor.matmul(
                        ps_v,
                        wpA_sb[:, base:base + 128],
                        wpA_sb[:, A_M0T + BPC * k:A_M0T + BPC * (k + 1)],
                        start=(k == 0),
                        stop=(k == 1),
                    )
                t = wp.tile([128, BPC], F32, name=f"v_sb{m}", bufs=1)
                nc.vector.tensor_scalar_add(
                    out=t, in0=ps_v, scalar1=wpA_sb[:, A_BVAL + m:A_BVAL + m + 1]
                )
                v_sb.append(t)

            # exp + fused sums (one PSUM accumulation group, M=40)
            for half in range(2):
                cols = slice(half * NQ, (half + 1) * NQ)
                nc.scalar.activation(out=E_act[:, cols], in_=ps_lgs[half],
                                     func=ACT.Exp,
                                     bias=qattn_sb[0:32, QA_BATTN:QA_BATTN + 1])
            ps_sum = ps.tile([40, NQ], F32, tag="sum", bufs=1)
            nc.tensor.matmul(ps_sum, qattn_sb[0:32, QA_G0:QA_G0 + 40],
                             E_act[:, 0:NQ], start=True, stop=False)
            nc.tensor.matmul(ps_sum, qattn_sb[0:32, QA_G1:QA_G1 + 40],
                             E_act[:, NQ:J], start=False, stop=True)
            rec = wp.tile([40, NQ], F32, bufs=1)
            nc.vector.reciprocal(out=rec, in_=ps_sum)
            s_sb = wp.tile([40, NQ], F32, bufs=1)
            nc.vector.tensor_mul(s_sb, ps_sum, rec)

            # ---- block-diagonal expansion (idle gpsimd), fused over b:
            # vd40_k col 32*b + 4*k + hl; U40 rows 32b+h hold U_b[h,:]
            vd_sb = []
            for k in range(2):
                t = wp.tile([128, 40], F32, name=f"vd_sb{k}", bufs=1)
                nc.gpsimd.memset(t, 0.0)
                for b in range(BPC):
                    for hl in range(4):
                        h = k * 4 + hl
                        rws = slice(32 * hl, 32 * (hl + 1))
                        nc.gpsimd.tensor_copy(
                            out=t[rws, 32 * b + h:32 * b + h + 1],
                            in_=v_sb[k][rws, b:b + 1],
                        )
                vd_sb.append(t)
            ps_U = ps.tile([40, D], F32, tag="U", bufs=1)
            nc.tensor.matmul(ps_U, vd_sb[0], wpB_sb[:, B_WOUT:B_WOUT + 256],
                             start=True, stop=False)
            nc.tensor.matmul(ps_U, vd_sb[1], wpB_sb[:, B_WOUT + 256:B_WOUT + 512],
                             start=False, stop=True)
            U_sb = wp.tile([40, D], F32, bufs=1)
            nc.scalar.activation(out=U_sb, in_=ps_U, func=ACT.Copy)

            # ---- final: out^T tiles [128, 300] = U_b[:,mc]^T @ s_b + b_out
            # out2 col layout: 600*m + 300*b + n  (m = c'/128 tile)
            for b in range(BPC):
                for m in range(2):
                    ps_o = ps.tile([128, NQ], F32, tag="o", bufs=3)
                    nc.tensor.matmul(
                        ps_o,
                        U_sb[32 * b:32 * b + 8, m * 128:(m + 1) * 128],
                        s_sb[32 * b:32 * b + 8, 0:NQ],
                        start=True,
                        stop=True,
                    )
                    o_sb = wp.tile([128, NQ], F32, tag="o_sb", bufs=4)
                    bias_ap = wpB_sb[:, B_BOUT + m:B_BOUT + m + 1]
                    if m == 0:
                        nc.scalar.activation(out=o_sb, in_=ps_o, func=ACT.Identity,
                                             bias=bias_ap)
                    else:
                        nc.vector.tensor_scalar_add(out=o_sb, in0=ps_o,
                                                    scalar1=bias_ap)
                    store_eng = nc.sync if (b + m) % 2 == 0 else nc.scalar
                    store_eng.dma_start(
                        out=out2[:, J * m + NQ * b:J * m + NQ * (b + 1)],
                        in_=o_sb,
                    )
    if split:
        _split_multiwaits(nc)
    return nc


def _get_bass() -> bass.Bass:
    if "nc" not in _BASS_CACHE:
        _BASS_CACHE["nc"] = _build_bass()
    return _BASS_CACHE["nc"]


def _kmajor(w):
    # [256, x] -> [128, 2*x] with columns x*k + c
    x = w.shape[1]
    return np.ascontiguousarray(
        w.reshape(2, 128, x).transpose(1, 0, 2).reshape(128, 2 * x)
    )


def _make_in_maps(query, memory, W_attn, b_attn, W_val, b_val, W_out, b_out):
    f = np.float32
    qT_full = query.astype(f, copy=False).transpose(2, 1, 0)  # [c, bs, nq]
    m0 = memory[0].astype(f, copy=False)                      # [bs, c]

    wA_base = np.zeros((128, A_COLS), f)
    wA_base[:, A_WVAL:A_WVAL + 512] = _kmajor(W_val.astype(f, copy=False))
    wA_base[:, A_BVAL:A_BVAL + 2] = b_val.astype(f, copy=False).reshape(2, 128).T

    wB = np.zeros((128, B_COLS), f)
    wB[:, B_WOUT:B_WOUT + 512] = _kmajor(W_out.astype(f, copy=False))
    wB[:, B_BOUT:B_BOUT + 2] = b_out.astype(f, copy=False).reshape(2, 128).T

    qattn_arr = np.zeros((128, QA_COLS), BF)
    qattn_arr[:, QA_WATTN:QA_WATTN + 64] = _kmajor(
        W_attn.astype(f, copy=False)
    ).astype(BF)
    G = np.repeat(np.eye(N_HEADS, dtype=f), N_POINTS, 0)  # [32, 8]
    g0 = np.zeros((32, 40), f)
    g0[:, 0:32] = np.tile(G, (1, 4))   # rows 8..31 duplicate h0 sums (finite)
    g1 = np.zeros((32, 40), f)
    g1[:, 32:40] = G
    qattn_arr[0:32, QA_G0:QA_G0 + 40] = g0.astype(BF)
    qattn_arr[0:32, QA_G1:QA_G1 + 40] = g1.astype(BF)
    qattn_arr[0:32, QA_BATTN] = b_attn.astype(f, copy=False).astype(BF)

    in_maps = []
    for c in range(N_CORES):
        bs_sl = slice(c * BPC, (c + 1) * BPC)
        wA = wA_base.copy()
        wA[:, A_M0T:A_M0T + 2 * BPC] = (
            m0[bs_sl, :].T.reshape(2, 128, BPC).transpose(1, 0, 2).reshape(128, 2 * BPC)
        )
        qc = np.ascontiguousarray(qT_full[:, bs_sl, :]).reshape(2, 128, BPC, NQ)
        q0 = np.empty((128, QD_COLS), BF)
        q0[:, 0:NQ] = qc[0, :, 0, :].astype(BF)
        q0[:, NQ:2 * NQ] = qc[1, :, 0, :].astype(BF)
        q1 = np.empty((128, QD_COLS), BF)
        q1[:, 0:NQ] = qc[0, :, 1, :].astype(BF)
        q1[:, NQ:2 * NQ] = qc[1, :, 1, :].astype(BF)
        in_maps.append({"wpA": wA, "wpB": wB, "qattn": qattn_arr,
                        "qd0": q0, "qd1": q1})
    return in_maps


def _get_exec():
    """Build the sharded PJRT executable once and reuse it across calls
    (run_bass_kernel_spmd re-jits on every invocation)."""
    if "exec" in _BASS_CACHE:
        return _BASS_CACHE["exec"]
    import jax
    from concourse import bass2jax

    nc = _get_bass()
    bass2jax.install_neuronx_cc_hook()
    assert nc.dbg_addr is None
    part_name = nc.partition_id_tensor.name if nc.partition_id_tensor else None
    in_names, out_names, out_avals = [], [], []
    for alloc in nc.m.functions[0].allocations:
        if not isinstance(alloc, mybir.MemoryLocationSet):
            continue
        name = alloc.memorylocations[0].name
        if alloc.kind == "ExternalInput":
            if name != part_name:
                in_names.append(name)
        elif alloc.kind == "ExternalOutput":
            out_names.append(name)
            out_avals.append(
                jax.core.ShapedArray(tuple(alloc.tensor_shape),
                                     mybir.dt.np(alloc.dtype))
            )
    n_params = len(in_names)
    all_names = in_names + out_names
    if part_name is not None:
        all_names.append(part_name)
    donate = tuple(range(n_params, n_params + len(out_names)))

    def _body(*args):
        operands = list(args)
        if part_name is not None:
            operands.append(bass2jax.partition_id_tensor())
        outs = bass2jax._bass_exec_p.bind(
            *operands,
            out_avals=tuple(out_avals),
            in_names=tuple(all_names),
            out_names=tuple(out_names),
            lowering_input_output_aliases=(),
            sim_require_finite=True,
            sim_require_nnan=True,
            nc=nc,
        )
        return tuple(outs)

    devices = jax.devices()[:N_CORES]
    mesh = bass2jax.Mesh(np.asarray(devices), ("core",))
    spec = (bass2jax.PartitionSpec("core"),)
    sharded = jax.jit(
        bass2jax.shard_map(
            _body, mesh=mesh,
            in_specs=spec * (n_params + len(out_names)),
            out_specs=spec * len(out_names),
            check_rep=False,
        ),
        donate_argnums=donate,
        keep_unused=True,
    )
    _BASS_CACHE["exec"] = (sharded, in_names, out_names, out_avals)
    return _BASS_CACHE["exec"]


def kernel(query, memory, W_attn, b_attn, W_val, b_val, W_out, b_out, **_unused):
    args = [np.asarray(a) for a in
            (query, memory, W_attn, b_attn, W_val, b_val, W_out, b_out)]
    in_maps = _make_in_maps(*args)
    sharded, in_names, out_names, out_avals = _get_exec()
    concat_in = [
        np.concatenate([in_maps[c][nm] for c in range(N_CORES)], axis=0)
        for nm in in_names
    ]
    concat_zeros = [
        np.zeros((N_CORES * av.shape[0], *av.shape[1:]), av.dtype)
        for av in out_avals
    ]
    out_arrs = sharded(*concat_in, *concat_zeros)
    o_all = np.asarray(out_arrs[0]).reshape(N_CORES, 128, 2, BPC, NQ)
    parts = [o_all[c].transpose(2, 3, 1, 0).reshape(BPC, NQ, D)
             for c in range(N_CORES)]
    full = np.concatenate(parts, axis=0).transpose(1, 0, 2)  # [nq, bs, c]
    return np.ascontiguousarray(full)



# revision 10
# speedup vs baseline: 1.3393x; 1.3393x over previous
"""Trainium2 Bass kernel for the DeformableDetr sparse-attention module.

Reference semantics (single device):
    q   = query.transpose(1,0,2)              # [bs, nq, c]
    attn = softmax((q @ W_attn + b_attn).reshape(bs,nq,H,P), -1)
    v    = memory[0] @ W_val + b_val          # only memory token 0 is live
    out  = (attn.sum(-1)[...,None] * v.reshape(bs,1,H,dh)).reshape(bs,nq,c)
    out  = out @ W_out + b_out
    return out.transpose(1,0,2)               # [nq, bs, c]

Algebraic structure: attn.sum(-1) is a softmax summed over its own axis,
which is identically 1 for ANY input (each softmax row sums to 1), so

    out[q, b, :] = (memory[0, b] @ W_val + b_val) @ W_out + b_out

independent of q -- the output is the [bs, c] row bank broadcast over all
300 queries.  The kernel computes that live math on device:

    ps_v[m]   = W_val[:, m-half]^T @ m0^T            (PE, k-split PSUM acc)
    v_sb      = ps_v + b_val                         (DVE, bf16)
    ps_row[m] = W_out[:, m-half]^T @ v               (PE, k-split PSUM acc)
    out tiles = broadcast(ps_row[m][:, b] + b_out)   (DVE/ACT fills)

and stores the full per-core output [128, 1200] bf16 with two concurrent
DMAs (SP + ACT).  Weights/inputs load as two bf16 panels on SP + ACT in
parallel.  bf16 end-to-end keeps the relative error ~1e-3, far inside
the 2e-2 gate.

This walrus build rejects instructions carrying more than one sync wait;
_split_multiwaits() legalizes the module by moving excess waits onto
same-engine InstNoOps placed directly before the instruction (the
in-order sequencer stalls on each semaphore in turn -- semantically
identical).

Sharding: data-parallel over batch, 2 batch elements per core x 8 cores.
"""

import sys

import numpy as np

sys.path.insert(0, "/opt/trn_rl_repo")

import ml_dtypes

import concourse.bass as bass
import concourse.tile as tile
from concourse import mybir
from concourse.bass import broadcast_tensor_aps

NQ, BS, NS, D = 300, 16, 13294, 256
N_CORES = 8
BPC = BS // N_CORES          # batch elements per core = 2
F32 = mybir.dt.float32
BF16 = mybir.dt.bfloat16
BF = ml_dtypes.bfloat16

# pa: bf16 value-projection panel [128, 648]
A_WVAL = 0                   # [128, 512], col 256k + c  (k-major W_val)
A_M0T = A_WVAL + 512         # [128, 4],   col 2k + b    (m0^T k-major)
A_BVROW = A_M0T + 2 * BPC    # rows 0..1:  pa[m, 516+c'] = b_val[128m+c']
A_SEL = A_BVROW + 128        # rows 0..1:  pa[k, 644+2m+b] = (k == m)
A_COLS = A_SEL + 2 * BPC     # = 648  (1296 B/partition, at the DMA floor)

# pb: bf16 output-projection panel [128, 644]
B_WOUT = 0                   # [128, 512], col 256k + c2 (k-major W_out)
B_BOROW = B_WOUT + 512       # rows 0..1:  pb[m, 512+c2] = b_out[128m+c2]
B_SEL = B_BOROW + 128        # rows 0..1:  pb[k, 640+2m+b] = (k == m)
B_COLS = B_SEL + 2 * BPC     # = 644

_BASS_CACHE: dict = {}


def _split_multiwaits(nc: bass.Bass) -> None:
    for fn in nc.m.functions:
        for blk in fn.blocks:
            out, changed = [], False
            for inst in blk.instructions:
                si = inst.sync_info
                if si is not None and len(si.on_wait) > 1:
                    waits = list(si.on_wait)
                    for i, w in enumerate(waits[:-1]):
                        out.append(
                            mybir.InstNoOp(
                                name=f"{inst.name}_prewait{i}",
                                engine=inst.engine,
                                bass_nofuse=True,
                                sync_info=mybir.SyncInfo(on_wait=[w], on_update=[]),
                            )
                        )
                    inst.sync_info = mybir.SyncInfo(
                        on_wait=[waits[-1]], on_update=list(si.on_update)
                    )
                    changed = True
                out.append(inst)
            if changed:
                blk.instructions = out


def _build_bass(split: bool = True) -> bass.Bass:
    nc = bass.Bass()
    pa = nc.declare_dram_parameter("pa", [128, A_COLS], BF16, isOutput=False)
    pb = nc.declare_dram_parameter("pb", [128, B_COLS], BF16, isOutput=False)
    out2 = nc.declare_dram_parameter("out2", [128, 4 * NQ], BF16, isOutput=True)

    ACT = mybir.ActivationFunctionType
    ADD = mybir.AluOpType.add

    with tile.TileContext(nc) as tc:
        with (
            tc.tile_pool(name="consts", bufs=1) as cp,
            tc.tile_pool(name="ps", bufs=1, space="PSUM") as ps,
        ):
            # ---- loads: SP carries pa (value path, needed first), ACT
            # carries pb; both DMAs run concurrently.
            pa_sb = cp.tile([128, A_COLS], BF16)
            nc.sync.dma_start(out=pa_sb, in_=pa[:, :])
            pb_sb = cp.tile([128, B_COLS], BF16)
            nc.scalar.dma_start(out=pb_sb, in_=pb[:, :])

            # zeros for the DVE fills (off critical path, Pool engine)
            zeros = cp.tile([128, NQ], BF16)
            nc.gpsimd.memset(zeros, 0.0)

            # ---- value projection: ps_v[m][p, b] = v[128m+p, b] + b_val
            # (bias rides the PSUM accumulation as a rank-1 matmul).
            ps_v = []
            for m in range(2):
                t = ps.tile([128, BPC], F32, tag=f"v{m}")
                nc.tensor.matmul(
                    t,
                    pa_sb[0:2, A_BVROW:A_BVROW + 128],
                    pa_sb[0:2, A_SEL + BPC * m:A_SEL + BPC * (m + 1)],
                    start=True,
                    stop=False,
                )
                for k in range(2):
                    nc.tensor.matmul(
                        t,
                        pa_sb[:, A_WVAL + 256 * k + 128 * m:
                              A_WVAL + 256 * k + 128 * (m + 1)],
                        pa_sb[:, A_M0T + BPC * k:A_M0T + BPC * (k + 1)],
                        start=False,
                        stop=(k == 1),
                    )
                ps_v.append(t)

            # v_sb[:, 2j:2j+2] = bf16(ps_v[j])   (DVE)
            v_sb = cp.tile([128, 2 * BPC], BF16)
            for j in range(2):
                nc.vector.tensor_copy(
                    out=v_sb[:, BPC * j:BPC * (j + 1)], in_=ps_v[j]
                )

            # ---- output projection: ps_row[m][p, b] = row[128m+p, b] + b_out
            # bias term first (no v_sb dep), then j-split accumulation so the
            # j=0 term issues as soon as v_sb half 0 lands.
            ps_row = []
            for m in range(2):
                t = ps.tile([128, BPC], F32, tag=f"r{m}")
                nc.tensor.matmul(
                    t,
                    pb_sb[0:2, B_BOROW:B_BOROW + 128],
                    pb_sb[0:2, B_SEL + BPC * m:B_SEL + BPC * (m + 1)],
                    start=True,
                    stop=False,
                )
                ps_row.append(t)
            for j in range(2):
                for m in range(2):
                    nc.tensor.matmul(
                        ps_row[m],
                        pb_sb[:, B_WOUT + 256 * j + 128 * m:
                              B_WOUT + 256 * j + 128 * (m + 1)],
                        v_sb[:, BPC * j:BPC * (j + 1)],
                        start=False,
                        stop=(j == 1),
                    )

            # ---- broadcast fills: out half m, cols 300b..300b+300 hold
            # row[:, m-half] for batch b (all 300 query columns identical).
            halves = [cp.tile([128, BPC * NQ], BF16, name=f"half{m}")
                      for m in range(2)]
            # DVE: (m,b) = (0,0), (0,1), (1,0) -- single-op fills
            for (m, b) in ((0, 0), (0, 1), (1, 0)):
                nc.vector.tensor_scalar_add(
                    out=halves[m][:, NQ * b:NQ * (b + 1)],
                    in0=zeros,
                    scalar1=ps_row[m][:, b:b + 1],
                )
            # ACT: (m,b) = (1,1): Identity(broadcast(ps_row col))
            out_ap = halves[1][:, NQ:2 * NQ]
            in_bc, _ = broadcast_tensor_aps(ps_row[1][:, 1:2], out_ap)
            nc.scalar.activation(out=out_ap, in_=in_bc, func=ACT.Identity)

            # ---- stores: two concurrent DMAs.
            nc.sync.dma_start(out=out2[:, 0:BPC * NQ], in_=halves[0])
            nc.scalar.dma_start(out=out2[:, BPC * NQ:2 * BPC * NQ], in_=halves[1])
    if split:
        _split_multiwaits(nc)
    return nc


def _get_bass() -> bass.Bass:
    if "nc" not in _BASS_CACHE:
        _BASS_CACHE["nc"] = _build_bass()
    return _BASS_CACHE["nc"]


def _kmajor(w):
    # [256, x] -> [128, 2*x] with columns x*k + c
    x = w.shape[1]
    return np.ascontiguousarray(
        w.reshape(2, 128, x).transpose(1, 0, 2).reshape(128, 2 * x)
    )


def _make_in_maps(query, memory, W_attn, b_attn, W_val, b_val, W_out, b_out):
    f = np.float32
    m0 = memory[0].astype(f, copy=False)                      # [bs, c]

    pa_base = np.zeros((128, A_COLS), BF)
    pa_base[:, A_WVAL:A_WVAL + 512] = _kmajor(W_val.astype(f, copy=False)).astype(BF)
    pa_base[0:2, A_BVROW:A_BVROW + 128] = b_val.astype(f, copy=False).reshape(2, 128).astype(BF)
    for m in range(2):
        pa_base[m, A_SEL + BPC * m:A_SEL + BPC * (m + 1)] = BF(1.0)

    pb_arr = np.zeros((128, B_COLS), BF)
    pb_arr[:, B_WOUT:B_WOUT + 512] = _kmajor(W_out.astype(f, copy=False)).astype(BF)
    pb_arr[0:2, B_BOROW:B_BOROW + 128] = b_out.astype(f, copy=False).reshape(2, 128).astype(BF)
    for m in range(2):
        pb_arr[m, B_SEL + BPC * m:B_SEL + BPC * (m + 1)] = BF(1.0)

    in_maps = []
    for c in range(N_CORES):
        m0c = m0[c * BPC:(c + 1) * BPC, :]                    # [BPC, 256]
        pa_arr = pa_base.copy()
        # col 2k + b = m0c[b, 128k + p]
        pa_arr[:, A_M0T:A_M0T + 2 * BPC] = (
            m0c.T.reshape(2, 128, BPC).transpose(1, 0, 2).reshape(128, 2 * BPC)
        ).astype(BF)
        in_maps.append({"pa": pa_arr, "pb": pb_arr})
    return in_maps


def _get_exec():
    """Build the sharded PJRT executable once and reuse it across calls
    (run_bass_kernel_spmd re-jits on every invocation)."""
    if "exec" in _BASS_CACHE:
        return _BASS_CACHE["exec"]
    import jax
    from concourse import bass2jax

    nc = _get_bass()
    bass2jax.install_neuronx_cc_hook()
    assert nc.dbg_addr is None
    part_name = nc.partition_id_tensor.name if nc.partition_id_tensor else None
    in_names, out_names, out_avals = [], [], []
    for alloc in nc.m.functions[0].allocations:
        if not isinstance(alloc, mybir.MemoryLocationSet):
            continue
        name = alloc.memorylocations[0].name
        if alloc.kind == "ExternalInput":
            if name != part_name:
                in_names.append(name)
        elif alloc.kind == "ExternalOutput":
            out_names.append(name)
            out_avals.append(
                jax.core.ShapedArray(tuple(alloc.tensor_shape),
                                     mybir.dt.np(alloc.dtype))
            )
    n_params = len(in_names)
    all_names = in_names + out_names
    if part_name is not None:
        all_names.append(part_name)
    donate = tuple(range(n_params, n_params + len(out_names)))

    def _body(*args):
        operands = list(args)
        if part_name is not None:
            operands.append(bass2jax.partition_id_tensor())
        outs = bass2jax._bass_exec_p.bind(
            *operands,
            out_avals=tuple(out_avals),
            in_names=tuple(all_names),
            out_names=tuple(out_names),
            lowering_input_output_aliases=(),
            sim_require_finite=True,
            sim_require_nnan=True,
            nc=nc,
        )
        return tuple(outs)

    devices = jax.devices()[:N_CORES]
    mesh = bass2jax.Mesh(np.asarray(devices), ("core",))
    spec = (bass2jax.PartitionSpec("core"),)
    sharded = jax.jit(
        bass2jax.shard_map(
            _body, mesh=mesh,
            in_specs=spec * (n_params + len(out_names)),
            out_specs=spec * len(out_names),
            check_rep=False,
        ),
        donate_argnums=donate,
        keep_unused=True,
    )
    _BASS_CACHE["exec"] = (sharded, in_names, out_names, out_avals)
    return _BASS_CACHE["exec"]


def kernel(query, memory, W_attn, b_attn, W_val, b_val, W_out, b_out, **_unused):
    args = [np.asarray(a) for a in
            (query, memory, W_attn, b_attn, W_val, b_val, W_out, b_out)]
    in_maps = _make_in_maps(*args)
    sharded, in_names, out_names, out_avals = _get_exec()
    concat_in = [
        np.concatenate([in_maps[c][nm] for c in range(N_CORES)], axis=0)
        for nm in in_names
    ]
    concat_zeros = [
        np.zeros((N_CORES * av.shape[0], *av.shape[1:]), av.dtype)
        for av in out_avals
    ]
    out_arrs = sharded(*concat_in, *concat_zeros)
    # out2[p, 600m + 300b + n] = out[n, bs0 + b, 128m + p]
    o_all = np.asarray(out_arrs[0]).astype(np.float32)
    o_all = o_all.reshape(N_CORES, 128, 2, BPC, NQ)
    parts = [o_all[c].transpose(2, 3, 1, 0).reshape(BPC, NQ, D)
             for c in range(N_CORES)]
    full = np.concatenate(parts, axis=0).transpose(1, 0, 2)  # [nq, bs, c]
    return np.ascontiguousarray(full)


# revision 15
# speedup vs baseline: 1.4147x; 1.0563x over previous
"""Trainium2 Bass kernel for the DeformableDetr sparse-attention module.

Reference semantics (single device):
    q   = query.transpose(1,0,2)              # [bs, nq, c]
    attn = softmax((q @ W_attn + b_attn).reshape(bs,nq,H,P), -1)
    v    = memory[0] @ W_val + b_val          # only memory token 0 is live
    out  = (attn.sum(-1)[...,None] * v.reshape(bs,1,H,dh)).reshape(bs,nq,c)
    out  = out @ W_out + b_out
    return out.transpose(1,0,2)               # [nq, bs, c]

Algebraic structure: attn.sum(-1) is a softmax summed over its own axis,
which is identically 1 for ANY input (each softmax row sums to 1), so

    out[q, b, :] = (memory[0, b] @ W_val + b_val) @ W_out + b_out

independent of q -- the output is the [bs, c] row bank broadcast over all
300 queries.  The kernel computes that live math on device:

    ps_v[m]   = W_val[:, m-half]^T @ m0^T            (PE, k-split PSUM acc)
    v_sb      = ps_v + b_val                         (DVE, bf16)
    ps_row[m] = W_out[:, m-half]^T @ v               (PE, k-split PSUM acc)
    out tiles = broadcast(ps_row[m][:, b] + b_out)   (DVE/ACT fills)

and stores the full per-core output [128, 1200] bf16 with two concurrent
DMAs (SP + ACT).  Weights/inputs load as two bf16 panels on SP + ACT in
parallel.  bf16 end-to-end keeps the relative error ~1e-3, far inside
the 2e-2 gate.

This walrus build rejects instructions carrying more than one sync wait;
_split_multiwaits() legalizes the module by moving excess waits onto
same-engine InstNoOps placed directly before the instruction (the
in-order sequencer stalls on each semaphore in turn -- semantically
identical).

Sharding: data-parallel over batch, 2 batch elements per core x 8 cores.
"""

import sys

import numpy as np

sys.path.insert(0, "/opt/trn_rl_repo")

import ml_dtypes

import concourse.bass as bass
import concourse.tile as tile
from concourse import mybir
from concourse.bass import broadcast_tensor_aps

NQ, BS, NS, D = 300, 16, 13294, 256
N_CORES = 8
BPC = BS // N_CORES          # batch elements per core = 2
F32 = mybir.dt.float32
BF16 = mybir.dt.bfloat16
BF = ml_dtypes.bfloat16

# pa: bf16 value-projection panel [128, 648]
A_WVAL = 0                   # [128, 512], col 256k + c  (k-major W_val)
A_M0T = A_WVAL + 512         # [128, 4],   col 2k + b    (m0^T k-major)
A_BVROW = A_M0T + 2 * BPC    # rows 0..1:  pa[m, 516+c'] = b_val[128m+c']
A_SEL = A_BVROW + 128        # rows 0..1:  pa[k, 644+2m+b] = (k == m)
A_COLS = A_SEL + 2 * BPC     # = 648  (1296 B/partition, at the DMA floor)

# pb: bf16 output-projection panel [128, 644]
B_WOUT = 0                   # [128, 512], col 256k + c2 (k-major W_out)
B_BOROW = B_WOUT + 512       # rows 0..1:  pb[m, 512+c2] = b_out[128m+c2]
B_SEL = B_BOROW + 128        # rows 0..1:  pb[k, 640+2m+b] = (k == m)
B_COLS = B_SEL + 2 * BPC     # = 644

_BASS_CACHE: dict = {}


def _split_multiwaits(nc: bass.Bass) -> None:
    for fn in nc.m.functions:
        for blk in fn.blocks:
            out, changed = [], False
            for inst in blk.instructions:
                si = inst.sync_info
                if si is not None and len(si.on_wait) > 1:
                    waits = list(si.on_wait)
                    for i, w in enumerate(waits[:-1]):
                        out.append(
                            mybir.InstNoOp(
                                name=f"{inst.name}_prewait{i}",
                                engine=inst.engine,
                                bass_nofuse=True,
                                sync_info=mybir.SyncInfo(on_wait=[w], on_update=[]),
                            )
                        )
                    inst.sync_info = mybir.SyncInfo(
                        on_wait=[waits[-1]], on_update=list(si.on_update)
                    )
                    changed = True
                out.append(inst)
            if changed:
                blk.instructions = out


def _build_bass(split: bool = True) -> bass.Bass:
    nc = bass.Bass()
    pa = nc.declare_dram_parameter("pa", [128, A_COLS], BF16, isOutput=False)
    pb = nc.declare_dram_parameter("pb", [128, B_COLS], BF16, isOutput=False)
    out2 = nc.declare_dram_parameter("out2", [128, 4 * NQ], BF16, isOutput=True)

    ACT = mybir.ActivationFunctionType
    ADD = mybir.AluOpType.add

    with tile.TileContext(nc) as tc:
        with (
            tc.tile_pool(name="consts", bufs=1) as cp,
            tc.tile_pool(name="ps", bufs=1, space="PSUM") as ps,
        ):
            # ---- loads: SP carries pa (value path, needed first), ACT
            # carries pb; both DMAs run concurrently.
            pa_sb = cp.tile([128, A_COLS], BF16)
            nc.sync.dma_start(out=pa_sb, in_=pa[:, :])
            pb_sb = cp.tile([128, B_COLS], BF16)
            nc.scalar.dma_start(out=pb_sb, in_=pb[:, :])

            # zeros for the DVE fills (off critical path, Pool engine)
            zeros = cp.tile([128, NQ // (2 * BPC)], BF16)
            nc.gpsimd.memset(zeros, 0.0)

            # ---- value projection: ps_v[:, 2m+b] = v[128m+p, b] + b_val
            # (bias rides the PSUM accumulation as a rank-1 matmul).
            ps_v = ps.tile([128, 2 * BPC], F32, tag="v")
            for m in range(2):
                sl = ps_v[:, BPC * m:BPC * (m + 1)]
                nc.tensor.matmul(
                    sl,
                    pa_sb[0:2, A_BVROW:A_BVROW + 128],
                    pa_sb[0:2, A_SEL + BPC * m:A_SEL + BPC * (m + 1)],
                    start=True,
                    stop=False,
                )
                for k in range(2):
                    nc.tensor.matmul(
                        sl,
                        pa_sb[:, A_WVAL + 256 * k + 128 * m:
                              A_WVAL + 256 * k + 128 * (m + 1)],
                        pa_sb[:, A_M0T + BPC * k:A_M0T + BPC * (k + 1)],
                        start=False,
                        stop=(k == 1),
                    )

            # v_sb = bf16(ps_v)   (DVE, one op)
            v_sb = cp.tile([128, 2 * BPC], BF16)
            nc.vector.tensor_copy(out=v_sb, in_=ps_v)

            # ---- output projection: ps_row[:, 2m+b] = row[128m+p, b] + b_out
            # (groups per m strictly sequential: one PSUM zero region)
            ps_row = ps.tile([128, 2 * BPC], F32, tag="r")
            for m in range(2):
                nc.tensor.matmul(
                    ps_row[:, BPC * m:BPC * (m + 1)],
                    pb_sb[0:2, B_BOROW:B_BOROW + 128],
                    pb_sb[0:2, B_SEL + BPC * m:B_SEL + BPC * (m + 1)],
                    start=True,
                    stop=False,
                )
                for j in range(2):
                    nc.tensor.matmul(
                        ps_row[:, BPC * m:BPC * (m + 1)],
                        pb_sb[:, B_WOUT + 256 * j + 128 * m:
                              B_WOUT + 256 * j + 128 * (m + 1)],
                        v_sb[:, BPC * j:BPC * (j + 1)],
                        start=False,
                        stop=(j == 1),
                    )

            # ---- broadcast bank: bank[:, 75t + g] = ps_row[:, t] for all g.
            # Stores then repeat the whole bank via a 0-stride OUTER AP dim
            # (fastest dim stays contiguous, 600B -- DGE-legal, no elem
            # penalty): out2[p, 300*rep + 75*t + g] = bank[p, 75*t + g].
            G = NQ // (2 * BPC)          # 75 columns per (m, b) block
            bank = cp.tile([128, NQ], BF16)
            for t in range(3):           # DVE fills
                nc.vector.tensor_scalar_add(
                    out=bank[:, G * t:G * (t + 1)],
                    in0=zeros,
                    scalar1=ps_row[:, t:t + 1],
                )
            # ACT fill for t=3: Copy of a broadcast (0-stride) engine AP
            out_ap = bank[:, 3 * G:4 * G]
            in_bc, _ = broadcast_tensor_aps(ps_row[:, 3:4], out_ap)
            nc.scalar.activation(out=out_ap, in_=in_bc, func=ACT.Copy)

            # ---- stores: two concurrent DMAs, each covering 2 repeats.
            for h in range(2):
                bsrc = bass.AP(
                    bank[:, :].tensor, bank[:, :].offset,
                    [list(bank[:, :].ap[0]), [0, 2], [1, NQ]],
                )
                eng = nc.sync if h == 0 else nc.scalar
                eng.dma_start(
                    out=out2[:, 2 * NQ * h:2 * NQ * (h + 1)], in_=bsrc
                )
    if split:
        _split_multiwaits(nc)
    return nc


def _get_bass() -> bass.Bass:
    if "nc" not in _BASS_CACHE:
        _BASS_CACHE["nc"] = _build_bass()
    return _BASS_CACHE["nc"]


def _kmajor(w):
    # [256, x] -> [128, 2*x] with columns x*k + c
    x = w.shape[1]
    return np.ascontiguousarray(
        w.reshape(2, 128, x).transpose(1, 0, 2).reshape(128, 2 * x)
    )


def _make_in_maps(query, memory, W_attn, b_attn, W_val, b_val, W_out, b_out):
    f = np.float32
    m0 = memory[0].astype(f, copy=False)                      # [bs, c]

    pa_base = np.zeros((128, A_COLS), BF)
    pa_base[:, A_WVAL:A_WVAL + 512] = _kmajor(W_val.astype(f, copy=False)).astype(BF)
    pa_base[0:2, A_BVROW:A_BVROW + 128] = b_val.astype(f, copy=False).reshape(2, 128).astype(BF)
    for m in range(2):
        pa_base[m, A_SEL + BPC * m:A_SEL + BPC * (m + 1)] = BF(1.0)

    pb_arr = np.zeros((128, B_COLS), BF)
    pb_arr[:, B_WOUT:B_WOUT + 512] = _kmajor(W_out.astype(f, copy=False)).astype(BF)
    pb_arr[0:2, B_BOROW:B_BOROW + 128] = b_out.astype(f, copy=False).reshape(2, 128).astype(BF)
    for m in range(2):
        pb_arr[m, B_SEL + BPC * m:B_SEL + BPC * (m + 1)] = BF(1.0)

    in_maps = []
    for c in range(N_CORES):
        m0c = m0[c * BPC:(c + 1) * BPC, :]                    # [BPC, 256]
        pa_arr = pa_base.copy()
        # col 2k + b = m0c[b, 128k + p]
        pa_arr[:, A_M0T:A_M0T + 2 * BPC] = (
            m0c.T.reshape(2, 128, BPC).transpose(1, 0, 2).reshape(128, 2 * BPC)
        ).astype(BF)
        in_maps.append({"pa": pa_arr, "pb": pb_arr})
    return in_maps


def _get_exec():
    """Build the sharded PJRT executable once and reuse it across calls
    (run_bass_kernel_spmd re-jits on every invocation)."""
    if "exec" in _BASS_CACHE:
        return _BASS_CACHE["exec"]
    import jax
    from concourse import bass2jax

    nc = _get_bass()
    bass2jax.install_neuronx_cc_hook()
    assert nc.dbg_addr is None
    part_name = nc.partition_id_tensor.name if nc.partition_id_tensor else None
    in_names, out_names, out_avals = [], [], []
    for alloc in nc.m.functions[0].allocations:
        if not isinstance(alloc, mybir.MemoryLocationSet):
            continue
        name = alloc.memorylocations[0].name
        if alloc.kind == "ExternalInput":
            if name != part_name:
                in_names.append(name)
        elif alloc.kind == "ExternalOutput":
            out_names.append(name)
            out_avals.append(
                jax.core.ShapedArray(tuple(alloc.tensor_shape),
                                     mybir.dt.np(alloc.dtype))
            )
    n_params = len(in_names)
    all_names = in_names + out_names
    if part_name is not None:
        all_names.append(part_name)
    donate = tuple(range(n_params, n_params + len(out_names)))

    def _body(*args):
        operands = list(args)
        if part_name is not None:
            operands.append(bass2jax.partition_id_tensor())
        outs = bass2jax._bass_exec_p.bind(
            *operands,
            out_avals=tuple(out_avals),
            in_names=tuple(all_names),
            out_names=tuple(out_names),
            lowering_input_output_aliases=(),
            sim_require_finite=True,
            sim_require_nnan=True,
            nc=nc,
        )
        return tuple(outs)

    devices = jax.devices()[:N_CORES]
    mesh = bass2jax.Mesh(np.asarray(devices), ("core",))
    spec = (bass2jax.PartitionSpec("core"),)
    sharded = jax.jit(
        bass2jax.shard_map(
            _body, mesh=mesh,
            in_specs=spec * (n_params + len(out_names)),
            out_specs=spec * len(out_names),
            check_rep=False,
        ),
        donate_argnums=donate,
        keep_unused=True,
    )
    _BASS_CACHE["exec"] = (sharded, in_names, out_names, out_avals)
    return _BASS_CACHE["exec"]


def kernel(query, memory, W_attn, b_attn, W_val, b_val, W_out, b_out, **_unused):
    args = [np.asarray(a) for a in
            (query, memory, W_attn, b_attn, W_val, b_val, W_out, b_out)]
    in_maps = _make_in_maps(*args)
    sharded, in_names, out_names, out_avals = _get_exec()
    concat_in = [
        np.concatenate([in_maps[c][nm] for c in range(N_CORES)], axis=0)
        for nm in in_names
    ]
    concat_zeros = [
        np.zeros((N_CORES * av.shape[0], *av.shape[1:]), av.dtype)
        for av in out_avals
    ]
    out_arrs = sharded(*concat_in, *concat_zeros)
    # out2[p, 300*rep + 75*(2m+b) + g] = out[75*rep + g, bs0 + b, 128m + p]
    G = NQ // (2 * BPC)
    o_all = np.asarray(out_arrs[0]).astype(np.float32)
    o_all = o_all.reshape(N_CORES, 128, 4, 2, BPC, G)  # [c, p, rep, m, b, g]
    parts = [o_all[c].transpose(3, 1, 4, 2, 0).reshape(BPC, NQ, D)
             for c in range(N_CORES)]
    full = np.concatenate(parts, axis=0).transpose(1, 0, 2)  # [nq, bs, c]
    return np.ascontiguousarray(full)


# revision 16
# speedup vs baseline: 1.8032x; 1.2747x over previous
"""Trainium2 Bass kernel for the DeformableDetr sparse-attention module.

Reference semantics (single device):
    q   = query.transpose(1,0,2)              # [bs, nq, c]
    attn = softmax((q @ W_attn + b_attn).reshape(bs,nq,H,P), -1)
    v    = memory[0] @ W_val + b_val          # only memory token 0 is live
    out  = (attn.sum(-1)[...,None] * v.reshape(bs,1,H,dh)).reshape(bs,nq,c)
    out  = out @ W_out + b_out
    return out.transpose(1,0,2)               # [nq, bs, c]

Algebraic structure: attn.sum(-1) is a softmax summed over its own axis,
which is identically 1 for ANY input (each softmax row sums to 1), so

    out[q, b, :] = (memory[0, b] @ W_val + b_val) @ W_out + b_out

independent of q -- the output is the [bs, c] row bank broadcast over all
300 queries.  The kernel computes that live math on device:

    ps_v[m]   = W_val[:, m-half]^T @ m0^T            (PE, k-split PSUM acc)
    v_sb      = ps_v + b_val                         (DVE, bf16)
    ps_row[m] = W_out[:, m-half]^T @ v               (PE, k-split PSUM acc)
    out tiles = broadcast(ps_row[m][:, b] + b_out)   (DVE/ACT fills)

and stores the full per-core output [128, 1200] bf16 with two concurrent
DMAs (SP + ACT).  Weights/inputs load as two bf16 panels on SP + ACT in
parallel.  bf16 end-to-end keeps the relative error ~1e-3, far inside
the 2e-2 gate.

This walrus build rejects instructions carrying more than one sync wait;
_split_multiwaits() legalizes the module by moving excess waits onto
same-engine InstNoOps placed directly before the instruction (the
in-order sequencer stalls on each semaphore in turn -- semantically
identical).

Sharding: data-parallel over batch, 2 batch elements per core x 8 cores.
"""

import sys

import numpy as np

sys.path.insert(0, "/opt/trn_rl_repo")

import ml_dtypes

import concourse.bass as bass
import concourse.tile as tile
from concourse import mybir
from concourse.bass import broadcast_tensor_aps

NQ, BS, NS, D = 300, 16, 13294, 256
N_CORES = 8
BPC = BS // N_CORES          # batch elements per core = 2
F32 = mybir.dt.float32
BF16 = mybir.dt.bfloat16
BF = ml_dtypes.bfloat16

# pa: bf16 value-projection panel [128, 648]
A_WVAL = 0                   # [128, 512], col 256k + c  (k-major W_val)
A_M0T = A_WVAL + 512         # [128, 4],   col 2k + b    (m0^T k-major)
A_BVROW = A_M0T + 2 * BPC    # rows 0..1:  pa[m, 516+c'] = b_val[128m+c']
A_SEL = A_BVROW + 128        # rows 0..1:  pa[k, 644+2m+b] = (k == m)
A_COLS = A_SEL + 2 * BPC     # = 648  (1296 B/partition, at the DMA floor)

# pb: bf16 output-projection panel [128, 644]
B_WOUT = 0                   # [128, 512], col 256k + c2 (k-major W_out)
B_BOROW = B_WOUT + 512       # rows 0..1:  pb[m, 512+c2] = b_out[128m+c2]
B_SEL = B_BOROW + 128        # rows 0..1:  pb[k, 640+2m+b] = (k == m)
B_COLS = B_SEL + 2 * BPC     # = 644

_BASS_CACHE: dict = {}


def _split_multiwaits(nc: bass.Bass) -> None:
    for fn in nc.m.functions:
        for blk in fn.blocks:
            out, changed = [], False
            for inst in blk.instructions:
                si = inst.sync_info
                if si is not None and len(si.on_wait) > 1:
                    waits = list(si.on_wait)
                    for i, w in enumerate(waits[:-1]):
                        out.append(
                            mybir.InstNoOp(
                                name=f"{inst.name}_prewait{i}",
                                engine=inst.engine,
                                bass_nofuse=True,
                                sync_info=mybir.SyncInfo(on_wait=[w], on_update=[]),
                            )
                        )
                    inst.sync_info = mybir.SyncInfo(
                        on_wait=[waits[-1]], on_update=list(si.on_update)
                    )
                    changed = True
                out.append(inst)
            if changed:
                blk.instructions = out


def _build_bass(split: bool = True) -> bass.Bass:
    nc = bass.Bass()
    pa = nc.declare_dram_parameter("pa", [128, A_COLS], BF16, isOutput=False)
    pb = nc.declare_dram_parameter("pb", [128, B_COLS], BF16, isOutput=False)
    out2 = nc.declare_dram_parameter("out2", [128, 4 * NQ], BF16, isOutput=True)

    ACT = mybir.ActivationFunctionType
    ADD = mybir.AluOpType.add

    with tile.TileContext(nc) as tc:
        with (
            tc.tile_pool(name="consts", bufs=1) as cp,
            tc.tile_pool(name="ps", bufs=1, space="PSUM") as ps,
        ):
            # ---- loads: SP carries pa (value path, needed first), ACT
            # carries pb; both DMAs run concurrently.
            pa_sb = cp.tile([128, A_COLS], BF16)
            nc.sync.dma_start(out=pa_sb, in_=pa[:, :])
            pb_sb = cp.tile([128, B_COLS], BF16)
            nc.scalar.dma_start(out=pb_sb, in_=pb[:, :])

            # zeros for the DVE fills (off critical path, Pool engine)
            zeros = cp.tile([128, NQ // (2 * BPC)], BF16)
            nc.gpsimd.memset(zeros, 0.0)

            # ---- value projection: ps_v[:, 2m+b] = v[128m+p, b] + b_val
            # (bias rides the PSUM accumulation as a rank-1 matmul).
            ps_v = ps.tile([128, 2 * BPC], F32, tag="v")
            for m in range(2):
                sl = ps_v[:, BPC * m:BPC * (m + 1)]
                nc.tensor.matmul(
                    sl,
                    pa_sb[0:2, A_BVROW:A_BVROW + 128],
                    pa_sb[0:2, A_SEL + BPC * m:A_SEL + BPC * (m + 1)],
                    start=True,
                    stop=False,
                )
                for k in range(2):
                    nc.tensor.matmul(
                        sl,
                        pa_sb[:, A_WVAL + 256 * k + 128 * m:
                              A_WVAL + 256 * k + 128 * (m + 1)],
                        pa_sb[:, A_M0T + BPC * k:A_M0T + BPC * (k + 1)],
                        start=False,
                        stop=(k == 1),
                    )

            # v_sb = bf16(ps_v)   (DVE, one op)
            v_sb = cp.tile([128, 2 * BPC], BF16)
            nc.vector.tensor_copy(out=v_sb, in_=ps_v)

            # ---- output projection: ps_row[:, 2m+b] = row[128m+p, b] + b_out
            # (groups per m strictly sequential: one PSUM zero region)
            ps_row = ps.tile([128, 2 * BPC], F32, tag="r")
            for m in range(2):
                nc.tensor.matmul(
                    ps_row[:, BPC * m:BPC * (m + 1)],
                    pb_sb[0:2, B_BOROW:B_BOROW + 128],
                    pb_sb[0:2, B_SEL + BPC * m:B_SEL + BPC * (m + 1)],
                    start=True,
                    stop=False,
                )
                for j in range(2):
                    nc.tensor.matmul(
                        ps_row[:, BPC * m:BPC * (m + 1)],
                        pb_sb[:, B_WOUT + 256 * j + 128 * m:
                              B_WOUT + 256 * j + 128 * (m + 1)],
                        v_sb[:, BPC * j:BPC * (j + 1)],
                        start=False,
                        stop=(j == 1),
                    )

            # ---- broadcast bank: bank[:, 75t + g] = ps_row[:, t] for all g.
            # Stores then repeat the whole bank via a 0-stride OUTER AP dim
            # (fastest dim stays contiguous, 600B -- DGE-legal, no elem
            # penalty): out2[p, 300*rep + 75*t + g] = bank[p, 75*t + g].
            G = NQ // (2 * BPC)          # 75 columns per (m, b) block
            bank = cp.tile([128, NQ], BF16)
            for t in range(4):           # DVE fills (4x bf16 mode, ~80ns each)
                nc.vector.tensor_scalar_add(
                    out=bank[:, G * t:G * (t + 1)],
                    in0=zeros,
                    scalar1=ps_row[:, t:t + 1],
                )

            # ---- stores: two concurrent DMAs, each covering 2 repeats.
            for h in range(2):
                bsrc = bass.AP(
                    bank[:, :].tensor, bank[:, :].offset,
                    [list(bank[:, :].ap[0]), [0, 2], [1, NQ]],
                )
                eng = nc.sync if h == 0 else nc.scalar
                eng.dma_start(
                    out=out2[:, 2 * NQ * h:2 * NQ * (h + 1)], in_=bsrc
                )
    if split:
        _split_multiwaits(nc)
    return nc


def _get_bass() -> bass.Bass:
    if "nc" not in _BASS_CACHE:
        _BASS_CACHE["nc"] = _build_bass()
    return _BASS_CACHE["nc"]


def _kmajor(w):
    # [256, x] -> [128, 2*x] with columns x*k + c
    x = w.shape[1]
    return np.ascontiguousarray(
        w.reshape(2, 128, x).transpose(1, 0, 2).reshape(128, 2 * x)
    )


def _make_in_maps(query, memory, W_attn, b_attn, W_val, b_val, W_out, b_out):
    f = np.float32
    m0 = memory[0].astype(f, copy=False)                      # [bs, c]

    pa_base = np.zeros((128, A_COLS), BF)
    pa_base[:, A_WVAL:A_WVAL + 512] = _kmajor(W_val.astype(f, copy=False)).astype(BF)
    pa_base[0:2, A_BVROW:A_BVROW + 128] = b_val.astype(f, copy=False).reshape(2, 128).astype(BF)
    for m in range(2):
        pa_base[m, A_SEL + BPC * m:A_SEL + BPC * (m + 1)] = BF(1.0)

    pb_arr = np.zeros((128, B_COLS), BF)
    pb_arr[:, B_WOUT:B_WOUT + 512] = _kmajor(W_out.astype(f, copy=False)).astype(BF)
    pb_arr[0:2, B_BOROW:B_BOROW + 128] = b_out.astype(f, copy=False).reshape(2, 128).astype(BF)
    for m in range(2):
        pb_arr[m, B_SEL + BPC * m:B_SEL + BPC * (m + 1)] = BF(1.0)

    in_maps = []
    for c in range(N_CORES):
        m0c = m0[c * BPC:(c + 1) * BPC, :]                    # [BPC, 256]
        pa_arr = pa_base.copy()
        # col 2k + b = m0c[b, 128k + p]
        pa_arr[:, A_M0T:A_M0T + 2 * BPC] = (
            m0c.T.reshape(2, 128, BPC).transpose(1, 0, 2).reshape(128, 2 * BPC)
        ).astype(BF)
        in_maps.append({"pa": pa_arr, "pb": pb_arr})
    return in_maps


def _get_exec():
    """Build the sharded PJRT executable once and reuse it across calls
    (run_bass_kernel_spmd re-jits on every invocation)."""
    if "exec" in _BASS_CACHE:
        return _BASS_CACHE["exec"]
    import jax
    from concourse import bass2jax

    nc = _get_bass()
    bass2jax.install_neuronx_cc_hook()
    assert nc.dbg_addr is None
    part_name = nc.partition_id_tensor.name if nc.partition_id_tensor else None
    in_names, out_names, out_avals = [], [], []
    for alloc in nc.m.functions[0].allocations:
        if not isinstance(alloc, mybir.MemoryLocationSet):
            continue
        name = alloc.memorylocations[0].name
        if alloc.kind == "ExternalInput":
            if name != part_name:
                in_names.append(name)
        elif alloc.kind == "ExternalOutput":
            out_names.append(name)
            out_avals.append(
                jax.core.ShapedArray(tuple(alloc.tensor_shape),
                                     mybir.dt.np(alloc.dtype))
            )
    n_params = len(in_names)
    all_names = in_names + out_names
    if part_name is not None:
        all_names.append(part_name)
    donate = tuple(range(n_params, n_params + len(out_names)))

    def _body(*args):
        operands = list(args)
        if part_name is not None:
            operands.append(bass2jax.partition_id_tensor())
        outs = bass2jax._bass_exec_p.bind(
            *operands,
            out_avals=tuple(out_avals),
            in_names=tuple(all_names),
            out_names=tuple(out_names),
            lowering_input_output_aliases=(),
            sim_require_finite=True,
            sim_require_nnan=True,
            nc=nc,
        )
        return tuple(outs)

    devices = jax.devices()[:N_CORES]
    mesh = bass2jax.Mesh(np.asarray(devices), ("core",))
    spec = (bass2jax.PartitionSpec("core"),)
    sharded = jax.jit(
        bass2jax.shard_map(
            _body, mesh=mesh,
            in_specs=spec * (n_params + len(out_names)),
            out_specs=spec * len(out_names),
            check_rep=False,
        ),
        donate_argnums=donate,
        keep_unused=True,
    )
    _BASS_CACHE["exec"] = (sharded, in_names, out_names, out_avals)
    return _BASS_CACHE["exec"]


def kernel(query, memory, W_attn, b_attn, W_val, b_val, W_out, b_out, **_unused):
    args = [np.asarray(a) for a in
            (query, memory, W_attn, b_attn, W_val, b_val, W_out, b_out)]
    in_maps = _make_in_maps(*args)
    sharded, in_names, out_names, out_avals = _get_exec()
    concat_in = [
        np.concatenate([in_maps[c][nm] for c in range(N_CORES)], axis=0)
        for nm in in_names
    ]
    concat_zeros = [
        np.zeros((N_CORES * av.shape[0], *av.shape[1:]), av.dtype)
        for av in out_avals
    ]
    out_arrs = sharded(*concat_in, *concat_zeros)
    # out2[p, 300*rep + 75*(2m+b) + g] = out[75*rep + g, bs0 + b, 128m + p]
    G = NQ // (2 * BPC)
    o_all = np.asarray(out_arrs[0]).astype(np.float32)
    o_all = o_all.reshape(N_CORES, 128, 4, 2, BPC, G)  # [c, p, rep, m, b, g]
    parts = [o_all[c].transpose(3, 1, 4, 2, 0).reshape(BPC, NQ, D)
             for c in range(N_CORES)]
    full = np.concatenate(parts, axis=0).transpose(1, 0, 2)  # [nq, bs, c]
    return np.ascontiguousarray(full)
